# revision 20
# baseline (speedup 1.0000x reference)
"""GAT (3-layer, 4-head) Trainium2 kernel — 8 NeuronCores, node-tile-parallel.

Algorithm (host prep + device):
  - Add self-loops; relabel nodes: sort by in-degree, tile into 392 tiles of
    128 nodes, deal tiles round-robin to 8 cores (device row = core-major).
  - Per node-tile padded CSR: each node's in-edge sources as slots on its
    partition; slots split into table-half A (rows < 32768) and B (rest)
    because dma_gather indices are int16.
  - Householder trick: rotate each head's 32-dim block so att_src becomes
    ||a||*e0.  Then per-edge alpha_src is channel h*32+0 of the gathered
    features, and alpha_dst = x @ (W^T att_dst_block) comes out of the
    phase-1 matmul as 4 extra columns.  Weighted sums run in the rotated
    basis; un-rotate with one matmul before bias+relu.
  - Per layer: per-tile matmul h~ = xT_tile @ [W~^T | Vdst] -> write own
    slice -> AllGather bf16 table -> per-tile dma_gather (A+B) -> small DVE
    ops for e/lrelu/exp/mask -> big DVE mult+reduce for messages -> 1/s
    normalize -> transpose+matmul (un-rotate) -> relu -> next layer.
  - softmax max-subtraction replaced by constant shift C (softmax-invariant;
    e range verified ~|e|<30 for this distribution, exp is f32-safe).
"""

import numpy as np

N = 50000
E = 800000
F = 128          # features (= H*C)
H = 4
CH = 32
L = 3
NC = 8           # cores
NPAD = 50176     # 392 tiles * 128
TILES = NPAD // 128
TPC = TILES // NC       # 49 tiles per core
RPC = NPAD // NC        # 6272 rows per core
SPLIT = 32768           # table half split (int16 index limit)
CSHIFT = 20.0           # constant exp shift (softmax invariant)
FOUT = 32


def _prep_graph(edge_index):
    """Returns (rowof[N]->device row, old_of_row[NPAD], per-core slot arrays).

    Slot arrays per core c:
      gidx  [128, 8*sumK] int16  (rows 0..15 hold wrapped indices)
      gmask [128, sumK]   f32
    plus shared per-k constants K_A[k], K_B[k] and offsets.
    """
    src = np.concatenate([edge_index[0].astype(np.int64), np.arange(N)])
    dst = np.concatenate([edge_index[1].astype(np.int64), np.arange(N)])
    deg = np.bincount(dst, minlength=N)
    order = np.argsort(-deg, kind="stable")          # sorted pos -> old node
    ii = np.arange(NPAD)
    t = ii // 128
    p = ii % 128
    row_of_pos = (t % NC) * RPC + (t // NC) * 128 + p
    rowof = np.empty(N, np.int64)
    rowof[order] = row_of_pos[:N]
    old_of_row = np.full(NPAD, -1, np.int64)
    old_of_row[rowof] = np.arange(N)

    srcr = rowof[src]
    dstr = rowof[dst]
    o = np.argsort(dstr, kind="stable")
    srcr_s = srcr[o]
    dstr_s = dstr[o]
    cnt = np.bincount(dstr_s, minlength=NPAD)
    off = np.concatenate([[0], np.cumsum(cnt)])

    # per-row A/B source lists
    nA = np.zeros(NPAD, np.int64)
    nB = np.zeros(NPAD, np.int64)
    srcsA = [None] * NPAD
    srcsB = [None] * NPAD
    for r in range(NPAD):
        s = srcr_s[off[r]:off[r + 1]]
        a = s[s < SPLIT]
        b = s[s >= SPLIT] - SPLIT
        srcsA[r] = a
        srcsB[r] = b
        nA[r] = len(a)
        nB[r] = len(b)

    # shared-per-k K values (max over cores, since the SPMD graph is shared)
    K_A = np.zeros(TPC, np.int64)
    K_B = np.zeros(TPC, np.int64)
    for c in range(NC):
        for k in range(TPC):
            r0 = c * RPC + k * 128
            K_A[k] = max(K_A[k], nA[r0:r0 + 128].max())
            K_B[k] = max(K_B[k], nB[r0:r0 + 128].max())
    K_A = np.maximum(K_A, 1)
    K_B = np.maximum(K_B, 1)
    K = K_A + K_B
    sumK = int(K.sum())

    gidx = np.zeros((NC, 128, 8 * sumK), np.int16)
    gmask = np.zeros((NC, 128, sumK), np.float32)
    oi = np.zeros(TPC + 1, np.int64)    # col offset into gidx (units of 8K)
    om = np.zeros(TPC + 1, np.int64)    # col offset into gmask (units of K)
    for k in range(TPC):
        oi[k + 1] = oi[k] + 8 * K[k]
        om[k + 1] = om[k] + K[k]

    for c in range(NC):
        for k in range(TPC):
            r0 = c * RPC + k * 128
            ka, kb = int(K_A[k]), int(K_B[k])
            kk = ka + kb
            lin = np.zeros(128 * kk, np.int16)
            msk = np.zeros((128, kk), np.float32)
            for pp in range(128):
                r = r0 + pp
                a = srcsA[r]
                b = srcsB[r]
                la, lb = len(a), len(b)
                if la:
                    lin[pp + 128 * np.arange(la)] = a
                    msk[pp, :la] = 1.0
                if lb:
                    lin[pp + 128 * (ka + np.arange(lb))] = b
                    msk[pp, ka:ka + lb] = 1.0
            wr = lin.reshape(-1, 16).T           # [16, 8*kk]
            gidx[c, :, oi[k]:oi[k] + 8 * kk] = np.tile(wr, (8, 1))
            gmask[c, :, om[k]:om[k] + kk] = msk

    return rowof, old_of_row, gidx, gmask, K_A, K_B, oi, om


def _prep_weights(Ws, att_src, att_dst, conv_bias, Wf, bf):
    """Householder-rotated weights.  Returns wcat [L,128,132], bmat [L,128,128],
    anorm [L,4], wfT [128,32]."""
    assert np.allclose(conv_bias, 0.0) and np.allclose(bf, 0.0), \
        "bias assumed zero (spec fill=zeros)"
    eye = np.eye(CH, dtype=np.float64)
    wcat = np.zeros((L, F, F + H), np.float32)
    bmat = np.zeros((L, F, F), np.float32)
    anorm = np.zeros((L, H), np.float32)
    for l in range(L):
        W = Ws[l].astype(np.float64)            # [F, F] (H*C, F_in)
        Bfull = np.zeros((F, F))
        for h in range(H):
            a = att_src[l, h].astype(np.float64)
            na = np.linalg.norm(a)
            anorm[l, h] = na
            if na < 1e-12:
                R = eye.copy()
            else:
                v = a.copy()
                v[0] -= na
                nv = np.linalg.norm(v)
                R = eye - 2.0 * np.outer(v, v) / (nv * nv) if nv > 1e-12 else eye.copy()
            Bfull[h * CH:(h + 1) * CH, h * CH:(h + 1) * CH] = R
        Wt = Bfull @ W                           # rotated W
        vcols = np.zeros((F, H))
        for h in range(H):
            blk = np.zeros(F)
            blk[h * CH:(h + 1) * CH] = att_dst[l, h]
            vcols[:, h] = W.T @ blk
        wcat[l] = np.concatenate([Wt.T, vcols], axis=1).astype(np.float32)
        bmat[l] = Bfull.astype(np.float32)
    wfT = Wf.T.astype(np.float32)               # [F, FOUT]
    return wcat, bmat, anorm, wfT


def _golden_device(x_dev, gidx, gmask, K_A, K_B, oi, om, wcat, bmat, anorm, wfT):
    """Numpy mirror of the device computation (same layouts & dtype casts)."""
    import ml_dtypes
    bf16 = ml_dtypes.bfloat16
    f32 = np.float32

    x_bf = x_dev.astype(bf16)                   # [NPAD, F]
    stats = []
    for l in range(L):
        wcat_bf = wcat[l].astype(bf16)
        had = x_bf.astype(f32) @ wcat_bf.astype(f32)     # [NPAD, 132]
        table = had[:, :F].astype(bf16)                  # allgathered
        adst = had[:, F:F + H].astype(f32)               # [NPAD, 4]
        x_new = np.zeros((NPAD, F), f32)
        emin, emax = 1e30, -1e30
        for c in range(NC):
            for k in range(TPC):
                r0 = c * RPC + k * 128
                ka, kb = int(K_A[k]), int(K_B[k])
                kk = ka + kb
                wr = gidx[c, :16, oi[k]:oi[k] + 8 * kk]
                lin = wr.T.reshape(-1).astype(np.int64)   # [128*kk]
                lin[128 * ka:] += SPLIT                   # B half abs rows
                G = table[lin].reshape(kk, 128, F).transpose(1, 0, 2)  # [128,kk,F]
                msk = gmask[c, :, om[k]:om[k] + kk]       # [128,kk]
                asrc = G[:, :, 0::CH].astype(f32)          # [128,kk,H]
                e = asrc * anorm[l][None, None, :] + adst[r0:r0 + 128][:, None, :]
                e = np.maximum(e, 0.2 * e)
                emin = min(emin, e.min())
                emax = max(emax, e.max())
                w = np.exp(e - CSHIFT).astype(bf16)
                w = (w * msk[:, :, None].astype(bf16)).astype(bf16)
                s = w.astype(f32).sum(axis=1)              # [128,H]
                rs = (1.0 / (s + 1e-16)).astype(f32)
                M = (G * w.reshape(128, kk, H, 1).repeat(CH, 3).reshape(128, kk, F)).astype(bf16)
                outrot = M.astype(f32).sum(axis=1)         # [128,F]
                outn = (outrot * rs.reshape(128, H, 1).repeat(CH, 2).reshape(128, F)).astype(bf16)
                xp = np.maximum(outn.astype(f32) @ bmat[l].astype(bf16).astype(f32), 0.0)
                x_new[r0:r0 + 128] = xp
        stats.append((emin, emax))
        x_bf = x_new.astype(bf16)
    out = x_bf.astype(f32) @ wfT.astype(bf16).astype(f32)  # [NPAD, FOUT]
    return out, stats


def _host_prep(inputs):
    x = inputs["x"]
    edge_index = inputs["edge_index"]
    rowof, old_of_row, gidx, gmask, K_A, K_B, oi, om = _prep_graph(edge_index)
    wcat, bmat, anorm, wfT = _prep_weights(
        inputs["Ws"], inputs["att_src"], inputs["att_dst"],
        inputs["conv_bias"], inputs["Wf"], inputs["bf"])
    x_dev = np.zeros((NPAD, F), np.float32)
    x_dev[rowof] = x
    return dict(rowof=rowof, old_of_row=old_of_row, gidx=gidx, gmask=gmask,
                K_A=K_A, K_B=K_B, oi=oi, om=om, wcat=wcat, bmat=bmat,
                anorm=anorm, wfT=wfT, x_dev=x_dev)


def kernel_golden(**inputs):
    """Pure-numpy end-to-end (for validation)."""
    pp = _host_prep(inputs)
    out_dev, stats = _golden_device(
        pp["x_dev"], pp["gidx"], pp["gmask"], pp["K_A"], pp["K_B"],
        pp["oi"], pp["om"], pp["wcat"], pp["bmat"], pp["anorm"], pp["wfT"])
    print("e ranges per layer:", stats)
    res = np.zeros((N, FOUT), np.float32)
    res = out_dev[pp["rowof"]]
    return res.astype(np.float32)


def _ap_view(base_ap, free_dims):
    """AP with same tensor/partition dim but custom free dims [(stride, n), ...]."""
    import concourse.bass as bass
    return bass.AP(
        tensor=base_ap.tensor,
        offset=base_ap.offset,
        ap=[list(base_ap.ap[0])] + [[s, n] for s, n in free_dims],
    )


def _build_bass(pp):
    import sys
    if "/opt/trn_rl_repo" not in sys.path:
        sys.path.insert(0, "/opt/trn_rl_repo")
    from contextlib import ExitStack
    import concourse.tile as tile
    from concourse import bass, mybir
    from concourse.bacc import Bacc
    from concourse.masks import make_identity

    K_A, K_B, oi, om = pp["K_A"], pp["K_B"], pp["oi"], pp["om"]
    K = K_A + K_B
    sumK = int(K.sum())
    NI = 8 * sumK
    f32 = mybir.dt.float32
    bf = mybir.dt.bfloat16
    i16 = mybir.dt.int16
    AX = mybir.AxisListType
    ALU = mybir.AluOpType
    ACTF = mybir.ActivationFunctionType

    nc = Bacc(None, num_devices=NC)
    xT_p = nc.declare_dram_parameter("xT", [F, RPC], f32, isOutput=False)
    gidx_p = nc.declare_dram_parameter("gidx", [128, NI], i16, isOutput=False)
    gmask_p = nc.declare_dram_parameter("gmask", [128, sumK], f32, isOutput=False)
    wcat_p = nc.declare_dram_parameter("wcat", [L, F, F + H], f32, isOutput=False)
    bmat_p = nc.declare_dram_parameter("bmat", [L, F, F], f32, isOutput=False)
    anorm_p = nc.declare_dram_parameter("anorm", [L, 128, H], f32, isOutput=False)
    wfT_p = nc.declare_dram_parameter("wfT", [F, FOUT], f32, isOutput=False)
    out_p = nc.declare_dram_parameter("out", [RPC, FOUT], f32, isOutput=True)

    hslice = nc.dram_tensor("hslice", [RPC, F], bf)
    table = nc.dram_tensor("table", [NPAD, F], bf, addr_space="Shared")

    ctx = ExitStack()
    with ctx:
        tc = ctx.enter_context(tile.TileContext(nc))
        persist = ctx.enter_context(tc.tile_pool(name="persist", bufs=1))
        wpool = ctx.enter_context(tc.tile_pool(name="wpool", bufs=1))
        gpool = ctx.enter_context(tc.tile_pool(name="gpool", bufs=3))
        mpool = ctx.enter_context(tc.tile_pool(name="mpool", bufs=2))
        spool = ctx.enter_context(tc.tile_pool(name="spool", bufs=3))
        pspool = ctx.enter_context(tc.tile_pool(name="pspool", bufs=2, space="PSUM"))
        pspool2 = ctx.enter_context(tc.tile_pool(name="pspool2", bufs=2, space="PSUM"))

        ident = persist.tile([128, 128], bf, name="ident")
        make_identity(nc, ident[:])
        nidx_reg = nc.gpsimd.alloc_register("nidx")
        negc = persist.tile([128, 1], f32, name="negc")
        nc.vector.memset(negc[:], -CSHIFT)
        zero_b = persist.tile([128, 1], f32, name="zero_b")
        nc.vector.memset(zero_b[:], 0.0)

        # resident weights / constants
        wcat_sb, bmat_sb, anorm_sb = [], [], []
        for l in range(L):
            wc_f = spool.tile([F, F + H], f32, name=f"wc_f{l}", tag="wcf")
            nc.sync.dma_start(out=wc_f[:], in_=wcat_p[l])
            wc = wpool.tile([F, F + H], bf, name=f"wcat{l}")
            nc.vector.tensor_copy(out=wc[:], in_=wc_f[:])
            wcat_sb.append(wc)
            bm_f = spool.tile([F, F], f32, name=f"bm_f{l}", tag="bmf")
            nc.sync.dma_start(out=bm_f[:], in_=bmat_p[l])
            bm = wpool.tile([F, F], bf, name=f"bmat{l}")
            nc.vector.tensor_copy(out=bm[:], in_=bm_f[:])
            bmat_sb.append(bm)
            an = wpool.tile([128, H], f32, name=f"anorm{l}")
            nc.sync.dma_start(out=an[:], in_=anorm_p[l])
            anorm_sb.append(an)
        wfT_f = spool.tile([F, FOUT], f32, name="wfT_f", tag="wcf")
        nc.sync.dma_start(out=wfT_f[:], in_=wfT_p[:])
        wfT_sb = wpool.tile([F, FOUT], bf, name="wfT")
        nc.vector.tensor_copy(out=wfT_sb[:], in_=wfT_f[:])

        # resident graph arrays
        gidx_sb = persist.tile([128, NI], i16, name="gidx_sb")
        nc.sync.dma_start(out=gidx_sb[:], in_=gidx_p[:])
        gmask_f = persist.tile([128, sumK], f32, name="gmask_f")
        nc.sync.dma_start(out=gmask_f[:], in_=gmask_p[:])
        gmask_sb = persist.tile([128, sumK], bf, name="gmask_sb")
        nc.vector.tensor_copy(out=gmask_sb[:], in_=gmask_f[:])

        # layer-0 xT (bf16, feat-major) per tile
        xT_tiles = []
        for k in range(TPC):
            xf = spool.tile([128, 128], f32, name="xf", tag="xf")
            nc.sync.dma_start(out=xf[:], in_=xT_p[:, k * 128:(k + 1) * 128])
            xt = persist.tile([128, 128], bf, name=f"xT{k}")
            nc.vector.tensor_copy(out=xt[:], in_=xf[:])
            xT_tiles.append(xt)

        adst_tiles = [persist.tile([128, H], f32, name=f"adst{k}") for k in range(TPC)]

        for l in range(L):
            # ---- phase 1: h~ = xT.T @ [W~^T | Vdst]  -> hslice, adst ----
            for k in range(TPC):
                ps1 = pspool.tile([128, F + H], f32, name="ps1", tag="ps1")
                nc.tensor.matmul(ps1[:], xT_tiles[k][:], wcat_sb[l][:],
                                 start=True, stop=True)
                h_sb = spool.tile([128, F], bf, name="h_sb", tag="h_sb")
                nc.vector.tensor_copy(out=h_sb[:], in_=ps1[:, :F])
                nc.vector.tensor_copy(out=adst_tiles[k][:], in_=ps1[:, F:F + H])
                nc.sync.dma_start(out=hslice[k * 128:(k + 1) * 128, :], in_=h_sb[:])

            nc.gpsimd.collective_compute(
                "AllGather", ALU.bypass,
                replica_groups=[list(range(NC))],
                ins=[hslice[:].opt()], outs=[table[:].opt()])

            # ---- per-tile slot phase ----
            for k in range(TPC):
                ka, kb = int(K_A[k]), int(K_B[k])
                kk = ka + kb
                G = gpool.tile([128, kk, F], bf, name="G", tag="G")
                nc.gpsimd.reg_mov(nidx_reg, 128 * ka)
                nc.gpsimd.dma_gather(
                    out_ap=G[:, :ka, :], in_ap=table[:SPLIT],
                    idxs_ap=gidx_sb[:, oi[k]:oi[k] + 8 * ka],
                    num_idxs=128 * ka, num_idxs_reg=nidx_reg, elem_size=F,
                    single_packet=False)
                nc.gpsimd.reg_mov(nidx_reg, 128 * kb)
                nc.gpsimd.dma_gather(
                    out_ap=G[:, ka:, :], in_ap=table[SPLIT:],
                    idxs_ap=gidx_sb[:, oi[k] + 8 * ka:oi[k] + 8 * kk],
                    num_idxs=128 * kb, num_idxs_reg=nidx_reg, elem_size=F,
                    single_packet=False)

                base = G[:, :, :]
                g_ch0 = _ap_view(base, [(F, kk), (CH, H)])       # [128,kk,H]
                e_f = spool.tile([128, kk, H], f32, name="e_f", tag="e_f")
                an_b = _ap_view(anorm_sb[l][:], [(0, kk), (1, H)])
                nc.vector.tensor_tensor(out=e_f[:], in0=g_ch0, in1=an_b,
                                        op=ALU.mult)
                ad_b = _ap_view(adst_tiles[k][:], [(0, kk), (1, H)])
                nc.vector.tensor_tensor(out=e_f[:], in0=e_f[:], in1=ad_b,
                                        op=ALU.add)
                # leaky relu on DVE: e = max(e, 0.2e)
                e2 = spool.tile([128, kk, H], f32, name="e2", tag="e2")
                nc.vector.tensor_scalar_mul(e2[:], e_f[:], 0.2)
                nc.vector.tensor_tensor(out=e_f[:], in0=e_f[:], in1=e2[:],
                                        op=ALU.max)
                w_bf = spool.tile([128, kk, H], bf, name="w_bf", tag="w_bf")
                nc.scalar.activation(out=w_bf[:], in_=e_f[:], func=ACTF.Exp,
                                     bias=negc[:], scale=1.0)
                mk_b = _ap_view(gmask_sb[:, om[k]:om[k] + kk], [(1, kk), (0, H)])
                nc.vector.tensor_tensor(out=w_bf[:], in0=w_bf[:], in1=mk_b,
                                        op=ALU.mult)
                # s = sum_j w  -> [128, H]
                s_f = spool.tile([128, H], f32, name="s_f", tag="s_f")
                w_hj = _ap_view(w_bf[:], [(1, H), (H, kk)])
                nc.vector.tensor_reduce(out=s_f[:], in_=w_hj, axis=AX.X,
                                        op=ALU.add)
                nc.vector.tensor_scalar_add(s_f[:], s_f[:], 1e-16)
                rs_f = spool.tile([128, H], f32, name="rs_f", tag="rs_f")
                nc.vector.reciprocal(out=rs_f[:], in_=s_f[:])
                # M = G * w  (broadcast over channel)
                M = mpool.tile([128, kk, F], bf, name="M", tag="M")
                w_b = _ap_view(w_bf[:], [(H, kk), (1, H), (0, CH)])
                g_4d = _ap_view(base, [(F, kk), (CH, H), (1, CH)])
                m_4d = _ap_view(M[:, :, :], [(F, kk), (CH, H), (1, CH)])
                nc.vector.tensor_tensor(out=m_4d, in0=g_4d, in1=w_b,
                                        op=ALU.mult)
                # outrot = sum_j M -> [128, F]
                orot = spool.tile([128, F], f32, name="orot", tag="orot")
                m_red = _ap_view(M[:, :, :], [(CH, H), (1, CH), (F, kk)])
                nc.vector.tensor_reduce(out=orot[:], in_=m_red, axis=AX.X,
                                        op=ALU.add)
                # normalize by 1/s, cast bf16
                on_bf = spool.tile([128, F], bf, name="on_bf", tag="on_bf")
                o_3d = _ap_view(orot[:], [(CH, H), (1, CH)])
                on_3d = _ap_view(on_bf[:], [(CH, H), (1, CH)])
                rs_b = _ap_view(rs_f[:], [(1, H), (0, CH)])
                nc.vector.tensor_tensor(out=on_3d, in0=o_3d, in1=rs_b,
                                        op=ALU.mult)
                # transpose, un-rotate (or final proj), relu
                ps_t = pspool2.tile([128, 128], bf, name="ps_t", tag="ps_t")
                nc.tensor.transpose(out=ps_t[:], in_=on_bf[:], identity=ident[:])
                onT = spool.tile([128, F], bf, name="onT", tag="on_bf")
                nc.vector.tensor_copy(out=onT[:], in_=ps_t[:])
                ps_x = pspool2.tile([128, F], f32, name="ps_x", tag="ps_x")
                nc.tensor.matmul(ps_x[:], onT[:], bmat_sb[l][:],
                                 start=True, stop=True)
                xnew = spool.tile([128, F], bf, name="xnew", tag="xnew")
                nc.scalar.activation(out=xnew[:], in_=ps_x[:], func=ACTF.Relu,
                                     bias=zero_b[:], scale=1.0)
                ps_t2 = pspool2.tile([128, 128], bf, name="ps_t2", tag="ps_t")
                nc.tensor.transpose(out=ps_t2[:], in_=xnew[:], identity=ident[:])
                nc.vector.tensor_copy(out=xT_tiles[k][:], in_=ps_t2[:])

        # ---- final projection ----
        for k in range(TPC):
            ps_o = pspool.tile([128, FOUT], f32, name="ps_o", tag="ps1")
            nc.tensor.matmul(ps_o[:], xT_tiles[k][:], wfT_sb[:],
                             start=True, stop=True)
            o_sb = spool.tile([128, FOUT], f32, name="o_sb", tag="o_sb")
            nc.vector.tensor_copy(out=o_sb[:], in_=ps_o[:])
            nc.sync.dma_start(out=out_p[k * 128:(k + 1) * 128, :], in_=o_sb[:])

    nc.finalize()
    return nc


TIME_ITERS = 0
LAST_TIMES = None


def _run_pjrt(nc, in_maps):
    """Execute the Bass module via PJRT (axon).  Mirrors
    bass2jax.run_bass_via_pjrt but keeps the jitted callable so repeated
    executions can be timed on pre-staged device buffers."""
    import jax
    import time
    from jax.sharding import Mesh, PartitionSpec
    from jax.experimental.shard_map import shard_map
    from concourse import bass2jax, mybir
    from concourse.bass2jax import _bass_exec_p, install_neuronx_cc_hook

    install_neuronx_cc_hook()
    n_cores = len(in_maps)
    partition_name = (nc.partition_id_tensor.name
                      if nc.partition_id_tensor else None)
    in_names, out_names, out_avals, zero_outs = [], [], [], []
    for alloc in nc.m.functions[0].allocations:
        if not isinstance(alloc, mybir.MemoryLocationSet):
            continue
        name = alloc.memorylocations[0].name
        if alloc.kind == "ExternalInput":
            if name != partition_name:
                in_names.append(name)
        elif alloc.kind == "ExternalOutput":
            out_names.append(name)
            shape = tuple(alloc.tensor_shape)
            dtype = mybir.dt.np(alloc.dtype)
            out_avals.append(jax.core.ShapedArray(shape, dtype))
            zero_outs.append(np.zeros(shape, dtype))
    n_params = len(in_names)
    n_outs = len(out_avals)
    in_names.extend(out_names)
    if partition_name is not None:
        in_names.append(partition_name)
    donate = tuple(range(n_params, n_params + n_outs))

    def _body(*args):
        operands = list(args)
        if partition_name is not None:
            operands.append(bass2jax.partition_id_tensor())
        outs = _bass_exec_p.bind(
            *operands, out_avals=tuple(out_avals), in_names=tuple(in_names),
            out_names=tuple(out_names), lowering_input_output_aliases=(),
            sim_require_finite=True, sim_require_nnan=True, nc=nc)
        return tuple(outs)

    devices = jax.devices()[:n_cores]
    mesh = Mesh(np.asarray(devices), ("core",))
    in_specs = (PartitionSpec("core"),) * (n_params + n_outs)
    out_specs = (PartitionSpec("core"),) * len(out_names)
    sharded = jax.jit(
        shard_map(_body, mesh=mesh, in_specs=in_specs, out_specs=out_specs,
                  check_rep=False),
        donate_argnums=donate, keep_unused=True)
    per_core = [[np.asarray(m[name]) for name in in_names[:n_params]]
                for m in in_maps]
    concat_in = [np.concatenate([per_core[c][i] for c in range(n_cores)], axis=0)
                 for i in range(n_params)]
    concat_zeros = [np.zeros((n_cores * z.shape[0], *z.shape[1:]), z.dtype)
                    for z in zero_outs]
    out_arrs = sharded(*concat_in, *concat_zeros)
    results = [
        {name: np.asarray(out_arrs[i]).reshape(n_cores, *out_avals[i].shape)[c]
         for i, name in enumerate(out_names)}
        for c in range(n_cores)]

    global LAST_TIMES
    LAST_TIMES = None
    if TIME_ITERS > 0:
        from jax.sharding import NamedSharding
        shardings = [NamedSharding(mesh, PartitionSpec("core"))] * n_params
        dev_in = jax.device_put(concat_in, shardings)
        times = []
        for _ in range(TIME_ITERS):
            zz = jax.device_put(
                concat_zeros,
                [NamedSharding(mesh, PartitionSpec("core"))] * n_outs)
            jax.block_until_ready(zz)
            t0 = time.perf_counter()
            o = sharded(*dev_in, *zz)
            jax.block_until_ready(o)
            times.append(time.perf_counter() - t0)
        LAST_TIMES = times
    return results


def kernel(**inputs):
    import sys
    if "/opt/trn_rl_repo" not in sys.path:
        sys.path.insert(0, "/opt/trn_rl_repo")

    inputs = {k: np.asarray(v) for k, v in inputs.items()}
    pp = _host_prep(inputs)
    nc = _build_bass(pp)

    anorm_rep = np.repeat(pp["anorm"][:, None, :], 128, axis=1).astype(np.float32)
    in_maps = []
    for c in range(NC):
        in_maps.append({
            "xT": np.ascontiguousarray(
                pp["x_dev"][c * RPC:(c + 1) * RPC].T).astype(np.float32),
            "gidx": np.ascontiguousarray(pp["gidx"][c]),
            "gmask": np.ascontiguousarray(pp["gmask"][c]),
            "wcat": pp["wcat"],
            "bmat": pp["bmat"],
            "anorm": anorm_rep,
            "wfT": pp["wfT"],
        })
    results = _run_pjrt(nc, in_maps)
    out_dev = np.concatenate([results[c]["out"] for c in range(NC)], axis=0)
    return out_dev[pp["rowof"]].astype(np.float32)


if __name__ == "__main__":
    pass
